# revision 1
# baseline (speedup 1.0000x reference)
"""HMM forward kernel v2 — time-segmented, latency-optimized.

Per core: 128 sequences x Ns=1024 steps, K=64 states.  Serial depth is cut
H-fold by splitting time into H segments: products of positive matrices
become rank-1 (Birkhoff contraction), so segment h>=1 only needs
  y_h = M_h @ 1      (forward chain from all-ones)
  w_h = M_h^T @ 1    (backward chain from all-ones)
and the total log-likelihood stitches with dot products:
  ll = C0 + sum_h Cf_h + ln(w_1.x0) + sum_h ln(w_h.y_{h-1}) + ln(1.y_{H-1})
       - sum_h ln(w_h.1)
(C* = logged renorm scales; backward scales cancel and are not logged.)

All 2H-1 chains advance in lockstep; ONE DVE tensor_tensor per slot does
every chain's elementwise P-multiply (amortizing the PSUM access penalty),
with two sequence-staggered groups (64 seqs each) to hide chain latency.

Layouts (per stagger-group gp):
  state  s   (128p, NCH, 32)  bf16   partition = 64*gs + k
  psum   v   (128p, NCH, 32)  f32    one bank
  chunk  P   (128p, NCH, 32*TCc) f32 per-c [b][t], exp'd in place
Chain order c: [seg0, fwd1..fwdH-1, bwd1..bwdH-1].
Backward chains use the pre-multiplied form w~_s = P_t(s) * (Tp @ w~_{s-1})
so every chain is matmul->multiply with the same slot alignment; host packs
backward P regions time-reversed.

Host packs "lp" as [gp][chunk i][gs][c][k][b][t] so every DMA is dense
2KB+ contiguous runs.
"""

from contextlib import ExitStack

import numpy as np
import ml_dtypes

import concourse.bass as bass
import concourse.tile as tile
from concourse import bacc, mybir

F32 = mybir.dt.float32
F16 = mybir.dt.float16
BF16 = mybir.dt.bfloat16
AFT = mybir.ActivationFunctionType

K = 64
CHAT = 0.5


def build_nc(ns=1024, h_seg=8, tc_chunk=16, r=64, lp_fp16=True,
             trn_type="TRN2"):
    S = ns // h_seg
    NCH = 2 * h_seg - 1
    n_chunks = S // tc_chunk
    nc = bacc.Bacc(trn_type, target_bir_lowering=False, debug=False)

    lp = nc.dram_tensor("lp", [2, n_chunks, NCH, 2, K, 32, tc_chunk],
                        F16 if lp_fp16 else F32, kind="ExternalInput")
    wts = nc.dram_tensor("wts", [3, 128, 128], BF16, kind="ExternalInput")
    cols = nc.dram_tensor("cols", [2, 128], F32, kind="ExternalInput")
    out_ll = nc.dram_tensor("ll", [2, 128, 32], F32, kind="ExternalOutput")

    with tile.TileContext(nc) as tc:
        with ExitStack() as ctx:
            _emit(ctx, tc, lp.ap(), wts.ap(), cols.ap(), out_ll.ap(),
                  S=S, H=h_seg, NCH=NCH, TCc=tc_chunk, n_chunks=n_chunks, r=r)
    nc.compile()
    return nc


def _emit(ctx, tc, lp, wts, cols, out_ll, *, S, H, NCH, TCc, n_chunks, r):
    nc = tc.nc
    BT = 32 * TCc

    consts = ctx.enter_context(tc.tile_pool(name="consts", bufs=1))
    pch_pools = [ctx.enter_context(tc.tile_pool(name=f"pch{g}", bufs=2))
                 for g in (0, 1)]
    s_pools = [ctx.enter_context(tc.tile_pool(name=f"s{g}", bufs=4))
               for g in (0, 1)]
    small = ctx.enter_context(tc.tile_pool(name="small", bufs=2))
    # v tile needs 2 PSUM banks when NCH>15; bufs=1 is safe (the WAR on the
    # bank coincides with the RAW chain through the state tile)
    vbufs = 3 if NCH <= 15 else 1
    v_pools = [ctx.enter_context(
        tc.tile_pool(name=f"v{g}", bufs=vbufs, space="PSUM"))
        for g in (0, 1)]
    z_psum = ctx.enter_context(tc.tile_pool(name="zp", bufs=1, space="PSUM"))

    # weights as three (128,128) lhsT tiles laid side by side on partitions 0..127
    w_t = consts.tile([128, 3, 128], BF16, name="wt3")
    nc.sync.dma_start(w_t[:, 0, :], wts[0])
    nc.sync.dma_start(w_t[:, 1, :], wts[1])
    nc.sync.dma_start(w_t[:, 2, :], wts[2])
    WF, WB, WZ = w_t[:, 0, :], w_t[:, 1, :], w_t[:, 2, :]

    cols_t = consts.tile([128, 2], F32, name="cols_t")
    nc.sync.dma_start(cols_t[:, :], cols.rearrange("c p -> p c"))
    PIP, TAU = cols_t[:, 0:1], cols_t[:, 1:2]
    warm = consts.tile([128, 1], F32, name="warm")
    nc.scalar.copy(warm[:, :], cols_t[:, 0:1])
    dwarm = consts.tile([128, 1], F32, name="dwarm")
    nc.vector.tensor_copy(dwarm[:, :], cols_t[:, 0:1])
    sd_t = consts.tile([1, 2, 3], F32, name="sd_t")
    nc.vector.memset(sd_t[:, :, :], 0.0)

    # z history: (128, NCH, 32, 2) slots: [0]=event z / recip(neg), [1]=stitch pos
    zh = [consts.tile([128, NCH, 32, 2], F32, name=f"zh{g}") for g in (0, 1)]
    for g in (0, 1):
        nc.vector.memset(zh[g][:, :, :, :], 1.0)

    csplit = [(c0, min(c0 + 2, NCH)) for c0 in range(0, NCH, 2)]

    PDT = lp.dtype

    def load_chunk(g, i):
        if s[g] is not None:
            # absorber: lets ACT observe the DVE tick that retires the chunk
            # buffer being reused, so the exps below don't carry a 3rd wait
            nc.scalar.copy(sd_t[0:1, g, 0:1], sd_t[0:1, g, 1:2])
        t_ = pch_pools[g].tile([128, NCH, 32, TCc], PDT, name="pch", tag="pch")
        for j, (c0, c1) in enumerate(csplit):
            # one DMA covers both partition halves -> the exp waits one sem
            eng = nc.sync if (g + j) % 2 == 0 else nc.gpsimd
            eng.dma_start(
                t_[:, c0:c1, :, :],
                lp[g, i, c0:c1].rearrange("c gs k b t -> (gs k) c b t"),
            )
            # CHAT is pre-subtracted on the host; bias=0 keeps deps minimal
            nc.scalar.activation(t_[:, c0:c1, :, :], t_[:, c0:c1, :, :],
                                 AFT.Exp)
        return t_

    s = [None, None]
    pch = [load_chunk(g, 0) for g in (0, 1)]

    # ---- slot 0: inits (read position t=0 of chunk 0) ----
    for g in (0, 1):
        s0 = s_pools[g].tile([128, NCH, 32], BF16, name="s", tag="s")
        p0 = pch[g][:, :, :, 0]        # (128, NCH, 32) position 0 slices
        nc.vector.tensor_scalar_mul(s0[:, 0, :], p0[:, 0, :], PIP)
        nc.vector.tensor_scalar_mul(s0[:, 1:H, :], p0[:, 1:H, :], TAU)
        nc.vector.tensor_copy(s0[:, H:NCH, :], p0[:, H:NCH, :])
        s[g] = s0

    def pe_absorb(t_dep):
        # ldweights reads the dependency tile: PE observes the producer's
        # tick without any tracked write, so following matmuls carry at most
        # one sync wait (the hardware MM limit)
        nc.tensor.ldweights(weights=t_dep)

    def zmm_all(g, s_cur, c0=0, c1=None):
        c1 = NCH if c1 is None else c1
        zb = z_psum.tile([128, NCH, 32], F32, name="zb", tag="zb")
        pe_absorb(s_cur[:, 0, 0:1])
        for c in range(c0, c1):
            nc.tensor.matmul(zb[:, c, :], lhsT=WZ, rhs=s_cur[:, c, :],
                             start=True, stop=True)
        return zb

    # ---- main slot loop (prefetch next chunk before stepping current) ----
    nxt = None
    for sig in range(1, S):
        i = sig // TCc
        if sig % TCc == 1 and i + 1 < n_chunks:
            nxt = [load_chunk(g, i + 1) for g in (0, 1)]
        if sig % TCc == 0 and i > 0:
            pch = nxt
        for g in (0, 1):
            v = v_pools[g].tile([128, NCH, 32], F32, name="v", tag="v")
            pe_absorb(s[g][:, 0, 0:1])
            for c in range(NCH):
                nc.tensor.matmul(v[:, c, :], lhsT=(WF if c < H else WB),
                                 rhs=s[g][:, c, :], start=True, stop=True)
            s_new = s_pools[g].tile([128, NCH, 32], BF16, name="s", tag="s")
            nc.vector.tensor_mul(s_new[:, :, :], v[:, :, :],
                                 pch[g][:, :, :, sig % TCc])
            s[g] = s_new

            if sig % TCc == TCc - 1:
                # retire marker: rides the s chain so its tick dominates every
                # reader of the finishing chunk (ACT absorber reads it later)
                nc.vector.tensor_copy(sd_t[0:1, g, 1:2], s[g][0:1, 0, 0:1])
                if nxt is not None:
                    # DVE observes the next chunk's exps before the boundary
                    # multiply so that multiply needs no 3rd sem wait
                    nc.vector.tensor_copy(sd_t[0:1, g, 2:3],
                                          nxt[g][0:1, NCH - 1, 0, 0:1])

            if sig % r == r - 1 and sig != S - 1:
                zb = zmm_all(g, s[g])
                rinv = small.tile([128, NCH, 32], F32, name="rinv", tag="rinv")
                nc.vector.reciprocal(rinv[:, :, :], zb[:, :, :])
                nc.vector.tensor_copy(zh[g][:, 0:H, :, 0], zb[:, 0:H, :])
                s_rn = s_pools[g].tile([128, NCH, 32], BF16, name="s", tag="s")
                nc.vector.tensor_mul(s_rn[:, :, :], s[g][:, :, :], rinv[:, :, :])
                s[g] = s_rn

    # ---- stitch ----
    for g in (0, 1):
        # bare backward matmuls: w_h = Tp @ w~_last
        wv = v_pools[g].tile([128, NCH, 32], F32, name="v", tag="v")
        pe_absorb(s[g][:, 0, 0:1])
        for c in range(H, NCH):
            nc.tensor.matmul(wv[:, c, :], lhsT=WB, rhs=s[g][:, c, :],
                             start=True, stop=True)
        wfin = small.tile([128, H - 1, 32], BF16, name="wfin", tag="wfin")
        nc.vector.tensor_copy(wfin[:, :, :], wv[:, H:NCH, :])
        # dots: w_h * x_{h-1}  (x-chain slices are exactly c=0..H-2)
        dprod = small.tile([128, H - 1, 32], BF16, name="dprod", tag="dprod")
        nc.vector.tensor_mul(dprod[:, :, :], wfin[:, :, :], s[g][:, 0:H - 1, :])
        # pos terms: colsum(dprod) for c=0..H-2, colsum(y_{H-1}) at c=H-1
        zp = z_psum.tile([128, NCH, 32], F32, name="zb", tag="zb")
        pe_absorb(dprod[:, 0, 0:1])
        for c in range(H - 1):
            nc.tensor.matmul(zp[:, c, :], lhsT=WZ, rhs=dprod[:, c, :],
                             start=True, stop=True)
        nc.tensor.matmul(zp[:, H - 1, :], lhsT=WZ, rhs=s[g][:, H - 1, :],
                         start=True, stop=True)
        # neg terms: colsum(w_h) -> store reciprocal (ln(1/x) = -ln x)
        for c in range(H, NCH):
            nc.tensor.matmul(zp[:, c, :], lhsT=WZ, rhs=wfin[:, c - H, :],
                             start=True, stop=True)
        nc.vector.tensor_copy(zh[g][:, 0:H, :, 1], zp[:, 0:H, :])
        nc.vector.reciprocal(zh[g][:, H:NCH, :, 0], zp[:, H:NCH, :])
        # ll = sum over (c, slot) of ln(zh)
        lnh = small.tile([128, NCH, 32, 2], F32, name="lnh", tag="lnh")
        nc.scalar.activation(lnh[:, :, :, :], zh[g][:, :, :, :], AFT.Ln)
        ll = small.tile([128, 32], F32, name="ll", tag="ll")
        lnh_bcs = bass.AP(tensor=lnh.tensor, offset=lnh.offset,
                          ap=[lnh.ap[0], [2, 32], [64, NCH], [1, 2]])
        nc.vector.tensor_reduce(ll[:, :], lnh_bcs, mybir.AxisListType.XY,
                                mybir.AluOpType.add)
        nc.sync.dma_start(out_ll[g, :, :], ll[:, :])


# ---------------- host side ----------------

def _log_softmax(x, axis):
    x = np.asarray(x, np.float64)
    m = x.max(axis=axis, keepdims=True)
    return x - m - np.log(np.exp(x - m).sum(axis=axis, keepdims=True))


def prep_inputs(log_pdf, pi, T, ns, h_seg=8, tc_chunk=16, n_cores=8,
                lp_fp16=True):
    Kd, N = log_pdf.shape
    b_total = N // ns
    b_core = b_total // n_cores
    S = ns // h_seg
    NCH = 2 * h_seg - 1
    n_chunks = S // tc_chunk

    logT = _log_softmax(T, 1)
    Tp = np.exp(logT)
    logpi = _log_softmax(pi, 0)

    wf = np.zeros((128, 128), np.float64)
    wf[:64, :64] = Tp; wf[64:, 64:] = Tp
    wb = np.zeros((128, 128), np.float64)
    wb[:64, :64] = Tp.T; wb[64:, 64:] = Tp.T
    wz = np.zeros((128, 128), np.float64)
    wz[:64, :64] = 1.0; wz[64:, 64:] = 1.0
    wts = np.stack([wf, wb, wz]).astype(ml_dtypes.bfloat16)

    pip = np.exp(logpi)                      # s0 init scalar
    tau = Tp.sum(axis=0)                     # Tp^T @ 1 per state
    cols = np.stack([np.concatenate([pip, pip]),
                     np.concatenate([tau, tau])]).astype(np.float32)  # (2,128)

    # P-position index maps per chain: pos p in [0,S) -> global t
    tmap = np.empty((NCH, S), np.int64)
    tmap[0] = np.arange(S)                                   # seg0
    for h in range(1, h_seg):
        tmap[h] = h * S + np.arange(S)                       # fwd h
        tmap[h_seg + h - 1] = (h + 1) * S - 1 - np.arange(S)  # bwd h (reversed)

    in_maps = []
    for core in range(n_cores):
        lp3 = log_pdf[:, core * b_core * ns: (core + 1) * b_core * ns]
        lp3 = np.asarray(lp3, np.float32).reshape(Kd, b_core, ns)  # [k,b,t]
        # gather [k, b, c, pos]
        gat = lp3[:, :, tmap.reshape(-1)].reshape(Kd, b_core, NCH, S) - CHAT
        # want [gp][i][gs][c][k][b32][t] ; b = 64*gp + 32*gs + b32
        gat = gat.reshape(Kd, 2, 2, 32, NCH, n_chunks, tc_chunk)
        pk = np.ascontiguousarray(gat.transpose(1, 5, 4, 2, 0, 3, 6),
                                  dtype=np.float16 if lp_fp16 else np.float32)
        in_maps.append({"lp": pk, "wts": wts, "cols": cols})
    return in_maps


def finish_output(results, ns):
    total = 0.0
    for res in results:
        ll = np.asarray(res["ll"], np.float64)  # (2,128,32)
        for g in (0, 1):
            for gs in (0, 1):
                total += (ll[g, 64 * gs, :] + ns * CHAT).sum()
    return np.float32(total)


# ---------------- harness entry point ----------------

_CACHED = {}


def _get_nc():
    if "nc" not in _CACHED:
        _CACHED["nc"] = build_nc(ns=1024, h_seg=8, tc_chunk=16, r=64)
    return _CACHED["nc"]


def kernel(log_pdf, pi, T, samples_per_sequence):
    """Full unsharded inputs -> full output (scalar f32), computed on 8
    TRN2 NeuronCores via the time-segmented scaled-forward kernel."""
    from concourse.bass_utils import run_bass_kernel_spmd

    ns = int(samples_per_sequence)
    assert log_pdf.shape == (64, 1048576) and ns == 1024, (
        "kernel is specialized to K=64, N=1048576, Ns=1024"
    )
    nc = _get_nc()
    in_maps = prep_inputs(np.asarray(log_pdf, np.float32),
                          np.asarray(pi, np.float32),
                          np.asarray(T, np.float32),
                          ns, h_seg=8, tc_chunk=16, n_cores=8)
    res = run_bass_kernel_spmd(nc, in_maps, core_ids=list(range(8)))
    return np.asarray(finish_output(res.results, ns), np.float32)



# revision 12
# speedup vs baseline: 1.4645x; 1.4645x over previous
"""HMM forward kernel v3 — host-exp, H=16 segments, truncated backward
chains, DVE+Pool multiply split, merged matmuls.

Per core: 128 sequences x Ns=1024 steps, K=64 states.  Time is split into
H=16 segments of S=64 steps; products of positive matrices contract to
rank-1 (Birkhoff), so
  ll = ln(x0 . y_1) + sum_{h=1..H-2} [ln(w_h . y_{h+1}) - ln(w_h . 1)]
with x0 the exact seg-0 forward state, y_h = M_h @ 1 forward chains, and
w_h = M_h^T @ 1 backward chains.  w_h only matters in DIRECTION (its scale
cancels), and the chain contracts per step, so the backward recursion is
truncated to the last jstar=8 factors of each segment — measured total
rel err ~1e-4 vs the 2e-2 gate.

Host-side prep computes P = exp(lp - CHAT) into fp16 (no on-chip exp; ACT
idle).  The per-slot elementwise P-multiply splits across DVE (fwd chains
0..DF, bwd 0..DB) and Pool/gpsimd (the rest) to beat DVE's 1x f32-PSUM
throughput wall.  Matmuls are merged per weight (one 512-col fwd matmul,
one 448-col bwd matmul) so PE.SEQ dispatch is off the critical path.

Layouts (per stagger-group g of 64 seqs; partition = 64*gs + k):
  state  s   (128p, 30, 32)  bf16  chains = [fwd 0..16 | bwd 16..30]
  psum   v   (128p, 30, 32)  f32   fwd->bank0 (2048B exact), bwd->bank1
  chunks     (128p, TCc, 16, 32) f16  t-major so per-slot slices are
             contiguous; DRAM is partition-major so each chunk is ONE DMA
             of 128 x (TCc*16*32*2)B contiguous runs.
Backward chains use the pre-multiplied form w~_s = P_t(s) * (Tp @ w~_{s-1})
(host packs their P time-reversed); after slot jstar-1 they finish with one
bare matmul w = Tp @ w~ absorbed into slot jstar, and the stitch dot
products run via block-ones colsum matmuls at the end, with ln + final sum
on the host (output = 29 z-values per sequence).
"""

from contextlib import ExitStack

import numpy as np
import ml_dtypes

import concourse.bass as bass
import concourse.tile as tile
from concourse import bacc, mybir

F32 = mybir.dt.float32
F16 = mybir.dt.float16
BF16 = mybir.dt.bfloat16

K = 64
CHAT = 0.5


def build_nc(ns=1024, h_seg=16, tc_chunk=8, jstar=8, df=10, db=9,
             trn_type="TRN2"):
    S = ns // h_seg
    NF = h_seg               # fwd chains incl seg0
    NB = h_seg - 2           # bwd chains (w_{H-1} cancels)
    NCH = NF + NB
    n_chunks = S // tc_chunk
    assert jstar <= tc_chunk, "bwd data must fit chunk 0"
    nc = bacc.Bacc(trn_type, target_bir_lowering=False, debug=False)

    lp_f = nc.dram_tensor("lpf", [2, n_chunks, 2, K, tc_chunk, NF, 32],
                          F16, kind="ExternalInput")
    lp_b = nc.dram_tensor("lpb", [2, 2, K, jstar, NB, 32],
                          F16, kind="ExternalInput")
    wts = nc.dram_tensor("wts", [3, 128, 128], BF16, kind="ExternalInput")
    cols = nc.dram_tensor("cols", [2, 128], F32, kind="ExternalInput")
    out_z = nc.dram_tensor("zz", [2, 2, NCH - 1, 32], F32,
                           kind="ExternalOutput")

    with tile.TileContext(nc) as tc:
        with ExitStack() as ctx:
            _emit(ctx, tc, lp_f.ap(), lp_b.ap(), wts.ap(), cols.ap(),
                  out_z.ap(), S=S, NF=NF, NB=NB, NCH=NCH, TCc=tc_chunk,
                  n_chunks=n_chunks, jstar=jstar, DF=df, DB=db)
    nc.compile()
    return nc


def _emit(ctx, tc, lp_f, lp_b, wts, cols, out_z, *, S, NF, NB, NCH, TCc,
          n_chunks, jstar, DF, DB):
    nc = tc.nc

    consts = ctx.enter_context(tc.tile_pool(name="consts", bufs=1))
    pchf_pools = [ctx.enter_context(tc.tile_pool(name=f"pchf{g}", bufs=2))
                  for g in (0, 1)]
    pchb_pools = [ctx.enter_context(tc.tile_pool(name=f"pchb{g}", bufs=1))
                  for g in (0, 1)]
    sd_pools = [ctx.enter_context(tc.tile_pool(name=f"sd{g}", bufs=4))
                for g in (0, 1)]
    sp_pools = [ctx.enter_context(tc.tile_pool(name=f"sp{g}", bufs=4))
                for g in (0, 1)]
    small = ctx.enter_context(tc.tile_pool(name="small", bufs=2))
    vd_pools = [ctx.enter_context(
        tc.tile_pool(name=f"vd{g}", bufs=1, space="PSUM"))
        for g in (0, 1)]
    vp_pools = [ctx.enter_context(
        tc.tile_pool(name=f"vp{g}", bufs=1, space="PSUM"))
        for g in (0, 1)]
    z_psum = ctx.enter_context(tc.tile_pool(name="zp", bufs=1, space="PSUM"))

    w_t = consts.tile([128, 3, 128], BF16, name="wt3")
    nc.sync.dma_start(w_t[:, 0, :], wts[0])
    nc.sync.dma_start(w_t[:, 1, :], wts[1])
    nc.sync.dma_start(w_t[:, 2, :], wts[2])
    WF, WB, WZ = w_t[:, 0, :], w_t[:, 1, :], w_t[:, 2, :]

    cols_t = consts.tile([128, 2], F32, name="cols_t")
    nc.sync.dma_start(cols_t[:, :], cols.rearrange("c p -> p c"))
    PIP, TAU = cols_t[:, 0:1], cols_t[:, 1:2]

    # finished backward vectors w_h = Tp @ w~ (written once at slot jstar)
    wfin = [consts.tile([128, NB, 32], BF16, name=f"wfin{g}") for g in (0, 1)]

    def load_fwd(g, i, t0, t1, t_=None):
        if t_ is None:
            t_ = pchf_pools[g].tile([128, TCc, NF, 32], F16, name="pchf",
                                    tag="pchf")
        eng = nc.sync if (g + i) % 2 == 0 else nc.scalar
        eng.dma_start(
            t_[:, t0:t1, :, :],
            lp_f[g, i, :, :, t0:t1].rearrange("gs k t c b -> (gs k) t c b"),
        )
        return t_

    # chunk-0 + bwd loads, split in t-halves so group 0 starts early
    hf = TCc // 2
    hb = jstar // 2
    pch = [None, None]
    pchb = [None, None]
    for g in (0, 1):
        pchb[g] = pchb_pools[g].tile([128, jstar, NB, 32], F16, name="pchb",
                                     tag="pchb")
    for t0, t1, b0, b1 in ((0, hf, 0, hb), (hf, TCc, hb, jstar)):
        for g in (0, 1):
            pch[g] = load_fwd(g, 0, t0, t1, t_=pch[g])
            eng = nc.scalar if g == 0 else nc.sync
            eng.dma_start(
                pchb[g][:, b0:b1, :, :],
                lp_b[g, :, :, b0:b1].rearrange("gs k t c b -> (gs k) t c b"),
            )

    # ---- slot 0: chain inits ----
    # Per-engine state ownership: DVE owns fwd chains 0:DF + bwd 0:DB in
    # s_d; Pool owns fwd DF:NF + bwd DB:NB in s_p.  No tile is ever
    # co-written by two engines, so each pipeline ping-pongs on one
    # semaphore pair with the PE and the groups overlap freely.
    PF = NF - DF
    PB = NB - DB
    sd = [None, None]
    sp = [None, None]
    for g in (0, 1):
        s0d = sd_pools[g].tile([128, DF + DB, 32], BF16, name="sd", tag="sd")
        p0 = pch[g][:, 0, :, :]
        nc.vector.tensor_scalar_mul(s0d[:, 0, :], p0[:, 0, :], PIP)
        nc.vector.tensor_scalar_mul(s0d[:, 1:DF, :], p0[:, 1:DF, :], TAU)
        nc.vector.tensor_copy(s0d[:, DF:DF + DB, :], pchb[g][:, 0, 0:DB, :])
        sd[g] = s0d
        s0p = sp_pools[g].tile([128, PF + PB, 32], BF16, name="sp", tag="sp")
        nc.gpsimd.tensor_scalar_mul(s0p[:, 0:PF, :], p0[:, DF:NF, :], TAU)
        nc.gpsimd.tensor_copy(s0p[:, PF:PF + PB, :], pchb[g][:, 0, DB:NB, :])
        sp[g] = s0p

    # ---- main slot loop ----
    # Matmuls are split by the engine that owns the chains (DVE: fwd 0:DF,
    # bwd NF:NF+DB; Pool: the rest) so every matmul and every multiply
    # carries exactly ONE inline sync wait — the two engine pipelines stay
    # fully decoupled and the stagger groups overlap cleanly.
    nxt = None
    for sig in range(1, S):
        i = sig // TCc
        t = sig % TCc
        if t == 1 and i + 1 < n_chunks:
            nxt = [load_fwd(g, i + 1, 0, TCc) for g in (0, 1)]
        if t == 0 and i > 0:
            pch = nxt
        bwd = sig < jstar
        vvd = [None, None]
        vvp = [None, None]
        for g in (0, 1):
            # v_d: fwd at [0:DF] (bank0), bwd at [16:16+DB] (bank1);
            # v_p: fwd+bwd packed in one bank
            v_d = vd_pools[g].tile([128, 25, 32], F32, name="vd", tag="vd")
            v_p = vp_pools[g].tile([128, PF + PB, 32], F32, name="vp",
                                   tag="vp")
            vvd[g], vvp[g] = v_d, v_p
            nc.tensor.matmul(v_d[:, 0:DF, :], lhsT=WF, rhs=sd[g][:, 0:DF, :],
                             start=True, stop=True)
            nc.tensor.matmul(v_p[:, 0:PF, :], lhsT=WF, rhs=sp[g][:, 0:PF, :],
                             start=True, stop=True)
        if bwd or sig == jstar:
            for g in (0, 1):
                # sig==jstar: bare finishing matmul w = Tp @ w~
                nc.tensor.matmul(vvd[g][:, 16:16 + DB, :], lhsT=WB,
                                 rhs=sd[g][:, DF:DF + DB, :], start=True,
                                 stop=True)
                nc.tensor.matmul(vvp[g][:, PF:PF + PB, :], lhsT=WB,
                                 rhs=sp[g][:, PF:PF + PB, :], start=True,
                                 stop=True)
        for g in (0, 1):
            v_d, v_p = vvd[g], vvp[g]
            s_nd = sd_pools[g].tile([128, DF + DB, 32], BF16, name="sd",
                                    tag="sd")
            s_np = sp_pools[g].tile([128, PF + PB, 32], BF16, name="sp",
                                    tag="sp")
            if bwd:
                nc.vector.tensor_mul(s_nd[:, DF:DF + DB, :],
                                     v_d[:, 16:16 + DB, :],
                                     pchb[g][:, sig, 0:DB, :])
                nc.vector.tensor_mul(s_np[:, PF:PF + PB, :],
                                     v_p[:, PF:PF + PB, :],
                                     pchb[g][:, sig, DB:NB, :])
            elif sig == jstar:
                nc.vector.tensor_copy(wfin[g][:, 0:DB, :],
                                      v_d[:, 16:16 + DB, :])
                nc.vector.tensor_copy(wfin[g][:, DB:NB, :],
                                      v_p[:, PF:PF + PB, :])
            nc.vector.tensor_mul(s_nd[:, 0:DF, :], v_d[:, 0:DF, :],
                                 pch[g][:, t, 0:DF, :])
            nc.vector.tensor_mul(s_np[:, 0:PF, :], v_p[:, 0:PF, :],
                                 pch[g][:, t, DF:NF, :])
            sd[g], sp[g] = s_nd, s_np

    # ---- stitch ----
    for g in (0, 1):
        dp = small.tile([128, NF - 1, 32], BF16, name="dp", tag="dp")
        nc.vector.tensor_mul(dp[:, 0:1, :], sd[g][:, 0:1, :],
                             sd[g][:, 1:2, :])
        nc.vector.tensor_mul(dp[:, 1:DF - 1, :], wfin[g][:, 0:DF - 2, :],
                             sd[g][:, 2:DF, :])
        nc.vector.tensor_mul(dp[:, DF - 1:NF - 1, :],
                             wfin[g][:, DF - 2:NB, :], sp[g][:, 0:PF, :])
        zz = z_psum.tile([128, 32, 32], F32, name="zz", tag="zz")
        nc.tensor.matmul(zz[:, 0:NF - 1, :], lhsT=WZ, rhs=dp[:, :, :],
                         start=True, stop=True)
        nc.tensor.matmul(zz[:, 16:16 + DB, :], lhsT=WZ,
                         rhs=wfin[g][:, 0:DB, :], start=True, stop=True)
        nc.tensor.matmul(zz[:, 16 + DB:16 + NB, :], lhsT=WZ,
                         rhs=wfin[g][:, DB:NB, :], start=True, stop=True)
        zs = small.tile([128, NCH - 1, 32], F32, name="zs", tag="zs")
        nc.vector.tensor_copy(zs[:, 0:NF - 1, :], zz[:, 0:NF - 1, :])
        nc.vector.tensor_copy(zs[:, NF - 1:NCH - 1, :], zz[:, 16:16 + NB, :])
        for gs in (0, 1):
            p0 = 64 * gs
            nc.sync.dma_start(out_z[g, gs:gs + 1, :, :],
                              zs[p0:p0 + 1, :, :])


# ---------------- host side ----------------

def _log_softmax(x, axis):
    x = np.asarray(x, np.float64)
    m = x.max(axis=axis, keepdims=True)
    return x - m - np.log(np.exp(x - m).sum(axis=axis, keepdims=True))


def prep_inputs(log_pdf, pi, T, ns, h_seg=16, tc_chunk=8, jstar=8,
                n_cores=8):
    Kd, N = log_pdf.shape
    b_total = N // ns
    b_core = b_total // n_cores
    S = ns // h_seg
    NF = h_seg
    NB = h_seg - 2
    n_chunks = S // tc_chunk

    logT = _log_softmax(T, 1)
    Tp = np.exp(logT)
    logpi = _log_softmax(pi, 0)

    wf = np.zeros((128, 128), np.float64)
    wf[:64, :64] = Tp; wf[64:, 64:] = Tp
    wb = np.zeros((128, 128), np.float64)
    wb[:64, :64] = Tp.T; wb[64:, 64:] = Tp.T
    wz = np.zeros((128, 128), np.float64)
    wz[:64, :64] = 1.0; wz[64:, 64:] = 1.0
    wts = np.stack([wf, wb, wz]).astype(ml_dtypes.bfloat16)

    pip = np.exp(logpi)
    tau = Tp.sum(axis=0)
    cols = np.stack([np.concatenate([pip, pip]),
                     np.concatenate([tau, tau])]).astype(np.float32)

    # host-side exp: P = exp(lp - CHAT) in fp16
    P_all = np.exp(np.asarray(log_pdf, np.float32) - CHAT).astype(np.float16)

    # bwd time map: chain c (seg h=c+1), slot sig -> t = (c+2)S - 1 - sig
    tb = (np.arange(NB)[:, None] + 2) * S - 1 - np.arange(jstar)[None, :]

    in_maps = []
    for core in range(n_cores):
        Pc = P_all[:, core * b_core * ns:(core + 1) * b_core * ns]
        Pc = Pc.reshape(Kd, b_core, ns)            # [k, b, t]
        # fwd: t = c*S + i*TCc + tt ; b = 64g + 32gs + b32
        v = Pc.reshape(Kd, 2, 2, 32, NF, n_chunks, tc_chunk)
        lpf = np.ascontiguousarray(v.transpose(1, 5, 2, 0, 6, 4, 3))
        # bwd gather: [k, g, gs, b32, c, sig] -> [g, gs, k, sig, c, b32]
        g2 = Pc.reshape(Kd, 2, 2, 32, ns)[:, :, :, :, tb]
        lpb = np.ascontiguousarray(g2.transpose(1, 2, 0, 5, 4, 3))
        in_maps.append({"lpf": lpf, "lpb": lpb, "wts": wts, "cols": cols})
    return in_maps


def finish_output(results, ns, h_seg=16):
    NF = h_seg
    total = 0.0
    for res in results:
        z = np.asarray(res["zz"], np.float64)      # [2, 2, 29, 32]
        lnz = np.log(z)
        total += lnz[:, :, 0:NF - 1, :].sum() - lnz[:, :, NF - 1:, :].sum()
        total += 128 * ns * CHAT
    return np.float32(total)


# ---------------- harness entry point ----------------

_CACHED = {}


def _get_nc():
    if "nc" not in _CACHED:
        _CACHED["nc"] = build_nc()
    return _CACHED["nc"]


def kernel(log_pdf, pi, T, samples_per_sequence):
    """Full unsharded inputs -> full output (scalar f32), computed on 8
    TRN2 NeuronCores via the time-segmented scaled-forward kernel."""
    from concourse.bass_utils import run_bass_kernel_spmd

    ns = int(samples_per_sequence)
    assert log_pdf.shape == (64, 1048576) and ns == 1024, (
        "kernel is specialized to K=64, N=1048576, Ns=1024"
    )
    nc = _get_nc()
    in_maps = prep_inputs(np.asarray(log_pdf, np.float32),
                          np.asarray(pi, np.float32),
                          np.asarray(T, np.float32),
                          ns, h_seg=16, tc_chunk=8, jstar=8, n_cores=8)
    res = run_bass_kernel_spmd(nc, in_maps, core_ids=list(range(8)))
    return np.asarray(finish_output(res.results, ns, h_seg=16), np.float32)


# revision 13
# speedup vs baseline: 1.5617x; 1.0664x over previous
"""HMM forward kernel v3 — host-exp, H=16 segments, truncated backward
chains, DVE+Pool multiply split, merged matmuls.

Per core: 128 sequences x Ns=1024 steps, K=64 states.  Time is split into
H=16 segments of S=64 steps; products of positive matrices contract to
rank-1 (Birkhoff), so
  ll = ln(x0 . y_1) + sum_{h=1..H-2} [ln(w_h . y_{h+1}) - ln(w_h . 1)]
with x0 the exact seg-0 forward state, y_h = M_h @ 1 forward chains, and
w_h = M_h^T @ 1 backward chains.  w_h only matters in DIRECTION (its scale
cancels), and the chain contracts per step, so the backward recursion is
truncated to the last jstar=8 factors of each segment — measured total
rel err ~1e-4 vs the 2e-2 gate.

Host-side prep computes P = exp(lp - CHAT) into fp16 (no on-chip exp; ACT
idle).  The per-slot elementwise P-multiply splits across DVE (fwd chains
0..DF, bwd 0..DB) and Pool/gpsimd (the rest) to beat DVE's 1x f32-PSUM
throughput wall.  Matmuls are merged per weight (one 512-col fwd matmul,
one 448-col bwd matmul) so PE.SEQ dispatch is off the critical path.

Layouts (per stagger-group g of 64 seqs; partition = 64*gs + k):
  state  s   (128p, 30, 32)  bf16  chains = [fwd 0..16 | bwd 16..30]
  psum   v   (128p, 30, 32)  f32   fwd->bank0 (2048B exact), bwd->bank1
  chunks     (128p, TCc, 16, 32) f16  t-major so per-slot slices are
             contiguous; DRAM is partition-major so each chunk is ONE DMA
             of 128 x (TCc*16*32*2)B contiguous runs.
Backward chains use the pre-multiplied form w~_s = P_t(s) * (Tp @ w~_{s-1})
(host packs their P time-reversed); after slot jstar-1 they finish with one
bare matmul w = Tp @ w~ absorbed into slot jstar, and the stitch dot
products run via block-ones colsum matmuls at the end, with ln + final sum
on the host (output = 29 z-values per sequence).
"""

from contextlib import ExitStack

import numpy as np
import ml_dtypes

import concourse.bass as bass
import concourse.tile as tile
from concourse import bacc, mybir

F32 = mybir.dt.float32
F16 = mybir.dt.float16
BF16 = mybir.dt.bfloat16

K = 64
CHAT = 0.5


def build_nc(ns=1024, h_seg=16, tc_chunk=8, jstar=8, df=12, db=10,
             trn_type="TRN2"):
    S = ns // h_seg
    NF = h_seg               # fwd chains incl seg0
    NB = h_seg - 2           # bwd chains (w_{H-1} cancels)
    NCH = NF + NB
    n_chunks = S // tc_chunk
    assert jstar <= tc_chunk, "bwd data must fit chunk 0"
    nc = bacc.Bacc(trn_type, target_bir_lowering=False, debug=False)

    lp_f = nc.dram_tensor("lpf", [2, n_chunks, 2, K, tc_chunk, NF, 32],
                          F16, kind="ExternalInput")
    lp_b = nc.dram_tensor("lpb", [2, 2, K, jstar, NB, 32],
                          F16, kind="ExternalInput")
    wts = nc.dram_tensor("wts", [3, 128, 128], BF16, kind="ExternalInput")
    cols = nc.dram_tensor("cols", [2, 128], F32, kind="ExternalInput")
    out_z = nc.dram_tensor("zz", [2, 2, NCH - 1, 32], F32,
                           kind="ExternalOutput")

    with tile.TileContext(nc) as tc:
        with ExitStack() as ctx:
            _emit(ctx, tc, lp_f.ap(), lp_b.ap(), wts.ap(), cols.ap(),
                  out_z.ap(), S=S, NF=NF, NB=NB, NCH=NCH, TCc=tc_chunk,
                  n_chunks=n_chunks, jstar=jstar, DF=df, DB=db)
    nc.compile()
    return nc


def _emit(ctx, tc, lp_f, lp_b, wts, cols, out_z, *, S, NF, NB, NCH, TCc,
          n_chunks, jstar, DF, DB):
    nc = tc.nc

    consts = ctx.enter_context(tc.tile_pool(name="consts", bufs=1))
    pchf_pools = [ctx.enter_context(tc.tile_pool(name=f"pchf{g}", bufs=2))
                  for g in (0, 1)]
    pchb_pools = [ctx.enter_context(tc.tile_pool(name=f"pchb{g}", bufs=1))
                  for g in (0, 1)]
    sd_pools = [ctx.enter_context(tc.tile_pool(name=f"sd{g}", bufs=4))
                for g in (0, 1)]
    sp_pools = [ctx.enter_context(tc.tile_pool(name=f"sp{g}", bufs=4))
                for g in (0, 1)]
    small = ctx.enter_context(tc.tile_pool(name="small", bufs=2))
    up_pools = [ctx.enter_context(tc.tile_pool(name=f"up{g}", bufs=2))
                for g in (0, 1)]
    vd_pools = [ctx.enter_context(
        tc.tile_pool(name=f"vd{g}", bufs=1, space="PSUM"))
        for g in (0, 1)]
    vp_pools = [ctx.enter_context(
        tc.tile_pool(name=f"vp{g}", bufs=1, space="PSUM"))
        for g in (0, 1)]
    z_psum = ctx.enter_context(tc.tile_pool(name="zp", bufs=1, space="PSUM"))

    w_t = consts.tile([128, 3, 128], BF16, name="wt3")
    nc.sync.dma_start(w_t[:, 0, :], wts[0])
    nc.sync.dma_start(w_t[:, 1, :], wts[1])
    nc.sync.dma_start(w_t[:, 2, :], wts[2])
    WF, WB, WZ = w_t[:, 0, :], w_t[:, 1, :], w_t[:, 2, :]

    cols_t = consts.tile([128, 2], F32, name="cols_t")
    nc.sync.dma_start(cols_t[:, :], cols.rearrange("c p -> p c"))
    PIP, TAU = cols_t[:, 0:1], cols_t[:, 1:2]

    # finished backward vectors w_h = Tp @ w~ (written once at slot jstar)
    wfin = [consts.tile([128, NB, 32], BF16, name=f"wfin{g}") for g in (0, 1)]

    def load_fwd(g, i, t0, t1, t_=None):
        if t_ is None:
            t_ = pchf_pools[g].tile([128, TCc, NF, 32], F16, name="pchf",
                                    tag="pchf")
        eng = nc.sync if (g + i) % 2 == 0 else nc.scalar
        eng.dma_start(
            t_[:, t0:t1, :, :],
            lp_f[g, i, :, :, t0:t1].rearrange("gs k t c b -> (gs k) t c b"),
        )
        return t_

    # chunk-0 + bwd loads, split in t-halves so group 0 starts early
    hf = TCc // 2
    hb = jstar // 2
    pch = [None, None]
    pchb = [None, None]
    for g in (0, 1):
        pchb[g] = pchb_pools[g].tile([128, jstar, NB, 32], F16, name="pchb",
                                     tag="pchb")
    for t0, t1, b0, b1 in ((0, hf, 0, hb), (hf, TCc, hb, jstar)):
        for g in (0, 1):
            pch[g] = load_fwd(g, 0, t0, t1, t_=pch[g])
            eng = nc.scalar if g == 0 else nc.sync
            eng.dma_start(
                pchb[g][:, b0:b1, :, :],
                lp_b[g, :, :, b0:b1].rearrange("gs k t c b -> (gs k) t c b"),
            )

    # ---- slot 0: chain inits ----
    # Per-engine state ownership: DVE owns fwd chains 0:DF + bwd 0:DB in
    # s_d; Pool owns fwd DF:NF + bwd DB:NB in s_p.  No tile is ever
    # co-written by two engines, so each pipeline ping-pongs on one
    # semaphore pair with the PE and the groups overlap freely.
    PF = NF - DF
    PB = NB - DB
    sd = [None, None]
    sp = [None, None]
    for g in (0, 1):
        s0d = sd_pools[g].tile([128, DF + DB, 32], BF16, name="sd", tag="sd")
        p0 = pch[g][:, 0, :, :]
        nc.vector.tensor_scalar_mul(s0d[:, 0, :], p0[:, 0, :], PIP)
        nc.vector.tensor_scalar_mul(s0d[:, 1:DF, :], p0[:, 1:DF, :], TAU)
        nc.vector.tensor_copy(s0d[:, DF:DF + DB, :], pchb[g][:, 0, 0:DB, :])
        sd[g] = s0d
        s0p = sp_pools[g].tile([128, PF + PB, 32], BF16, name="sp", tag="sp")
        nc.gpsimd.tensor_scalar_mul(s0p[:, 0:PF, :], p0[:, DF:NF, :], TAU)
        nc.gpsimd.tensor_copy(s0p[:, PF:PF + PB, :], pchb[g][:, 0, DB:NB, :])
        sp[g] = s0p

    # ---- main slot loop ----
    # Matmuls are split by the engine that owns the chains (DVE: fwd 0:DF,
    # bwd NF:NF+DB; Pool: the rest) so every matmul and every multiply
    # carries exactly ONE inline sync wait — the two engine pipelines stay
    # fully decoupled and the stagger groups overlap cleanly.
    nxt = None
    for sig in range(1, S):
        i = sig // TCc
        t = sig % TCc
        if t == 1 and i + 1 < n_chunks:
            nxt = [load_fwd(g, i + 1, 0, TCc) for g in (0, 1)]
        if t == 0 and i > 0:
            pch = nxt
        bwd = sig < jstar
        vvd = [None, None]
        vvp = [None, None]
        for g in (0, 1):
            # v_d: fwd at [0:DF] (bank0), bwd at [16:16+DB] (bank1);
            # v_p: fwd+bwd packed in one bank
            v_d = vd_pools[g].tile([128, 26, 32], F32, name="vd", tag="vd")
            v_p = vp_pools[g].tile([128, PF + PB, 32], F32, name="vp",
                                   tag="vp")
            vvd[g], vvp[g] = v_d, v_p
            nc.tensor.matmul(v_d[:, 0:DF, :], lhsT=WF, rhs=sd[g][:, 0:DF, :],
                             start=True, stop=True)
            nc.tensor.matmul(v_p[:, 0:PF, :], lhsT=WF, rhs=sp[g][:, 0:PF, :],
                             start=True, stop=True)
        if bwd or sig == jstar:
            for g in (0, 1):
                # sig==jstar: bare finishing matmul w = Tp @ w~
                nc.tensor.matmul(vvd[g][:, 16:16 + DB, :], lhsT=WB,
                                 rhs=sd[g][:, DF:DF + DB, :], start=True,
                                 stop=True)
                nc.tensor.matmul(vvp[g][:, PF:PF + PB, :], lhsT=WB,
                                 rhs=sp[g][:, PF:PF + PB, :], start=True,
                                 stop=True)
        for g in (0, 1):
            v_d, v_p = vvd[g], vvp[g]
            s_nd = sd_pools[g].tile([128, DF + DB, 32], BF16, name="sd",
                                    tag="sd")
            s_np = sp_pools[g].tile([128, PF + PB, 32], BF16, name="sp",
                                    tag="sp")
            # ACT stages v_p into SBUF (GPSIMD cannot read PSUM); Pool
            # multiplies SBUF-only, keeping those chains off DVE
            u_p = up_pools[g].tile([128, PF + PB, 32], BF16, name="up",
                                   tag="up")
            ce = PF + PB if bwd else PF
            nc.scalar.copy(u_p[:, 0:ce, :], v_p[:, 0:ce, :])
            if bwd:
                nc.vector.tensor_mul(s_nd[:, DF:DF + DB, :],
                                     v_d[:, 16:16 + DB, :],
                                     pchb[g][:, sig, 0:DB, :])
                nc.gpsimd.tensor_mul(s_np[:, PF:PF + PB, :],
                                     u_p[:, PF:PF + PB, :],
                                     pchb[g][:, sig, DB:NB, :])
            elif sig == jstar:
                nc.vector.tensor_copy(wfin[g][:, 0:DB, :],
                                      v_d[:, 16:16 + DB, :])
                nc.vector.tensor_copy(wfin[g][:, DB:NB, :],
                                      v_p[:, PF:PF + PB, :])
            nc.vector.tensor_mul(s_nd[:, 0:DF, :], v_d[:, 0:DF, :],
                                 pch[g][:, t, 0:DF, :])
            nc.gpsimd.tensor_mul(s_np[:, 0:PF, :], u_p[:, 0:PF, :],
                                 pch[g][:, t, DF:NF, :])
            sd[g], sp[g] = s_nd, s_np

    # ---- stitch ----
    for g in (0, 1):
        dp = small.tile([128, NF - 1, 32], BF16, name="dp", tag="dp")
        nc.vector.tensor_mul(dp[:, 0:1, :], sd[g][:, 0:1, :],
                             sd[g][:, 1:2, :])
        nc.vector.tensor_mul(dp[:, 1:DF - 1, :], wfin[g][:, 0:DF - 2, :],
                             sd[g][:, 2:DF, :])
        nc.vector.tensor_mul(dp[:, DF - 1:NF - 1, :],
                             wfin[g][:, DF - 2:NB, :], sp[g][:, 0:PF, :])
        zz = z_psum.tile([128, 32, 32], F32, name="zz", tag="zz")
        nc.tensor.matmul(zz[:, 0:NF - 1, :], lhsT=WZ, rhs=dp[:, :, :],
                         start=True, stop=True)
        nc.tensor.matmul(zz[:, 16:16 + DB, :], lhsT=WZ,
                         rhs=wfin[g][:, 0:DB, :], start=True, stop=True)
        nc.tensor.matmul(zz[:, 16 + DB:16 + NB, :], lhsT=WZ,
                         rhs=wfin[g][:, DB:NB, :], start=True, stop=True)
        zs = small.tile([128, NCH - 1, 32], F32, name="zs", tag="zs")
        nc.vector.tensor_copy(zs[:, 0:NF - 1, :], zz[:, 0:NF - 1, :])
        nc.vector.tensor_copy(zs[:, NF - 1:NCH - 1, :], zz[:, 16:16 + NB, :])
        for gs in (0, 1):
            p0 = 64 * gs
            nc.sync.dma_start(out_z[g, gs:gs + 1, :, :],
                              zs[p0:p0 + 1, :, :])


# ---------------- host side ----------------

def _log_softmax(x, axis):
    x = np.asarray(x, np.float64)
    m = x.max(axis=axis, keepdims=True)
    return x - m - np.log(np.exp(x - m).sum(axis=axis, keepdims=True))


def prep_inputs(log_pdf, pi, T, ns, h_seg=16, tc_chunk=8, jstar=8,
                n_cores=8):
    Kd, N = log_pdf.shape
    b_total = N // ns
    b_core = b_total // n_cores
    S = ns // h_seg
    NF = h_seg
    NB = h_seg - 2
    n_chunks = S // tc_chunk

    logT = _log_softmax(T, 1)
    Tp = np.exp(logT)
    logpi = _log_softmax(pi, 0)

    wf = np.zeros((128, 128), np.float64)
    wf[:64, :64] = Tp; wf[64:, 64:] = Tp
    wb = np.zeros((128, 128), np.float64)
    wb[:64, :64] = Tp.T; wb[64:, 64:] = Tp.T
    wz = np.zeros((128, 128), np.float64)
    wz[:64, :64] = 1.0; wz[64:, 64:] = 1.0
    wts = np.stack([wf, wb, wz]).astype(ml_dtypes.bfloat16)

    pip = np.exp(logpi)
    tau = Tp.sum(axis=0)
    cols = np.stack([np.concatenate([pip, pip]),
                     np.concatenate([tau, tau])]).astype(np.float32)

    # host-side exp: P = exp(lp - CHAT) in fp16
    P_all = np.exp(np.asarray(log_pdf, np.float32) - CHAT).astype(np.float16)

    # bwd time map: chain c (seg h=c+1), slot sig -> t = (c+2)S - 1 - sig
    tb = (np.arange(NB)[:, None] + 2) * S - 1 - np.arange(jstar)[None, :]

    in_maps = []
    for core in range(n_cores):
        Pc = P_all[:, core * b_core * ns:(core + 1) * b_core * ns]
        Pc = Pc.reshape(Kd, b_core, ns)            # [k, b, t]
        # fwd: t = c*S + i*TCc + tt ; b = 64g + 32gs + b32
        v = Pc.reshape(Kd, 2, 2, 32, NF, n_chunks, tc_chunk)
        lpf = np.ascontiguousarray(v.transpose(1, 5, 2, 0, 6, 4, 3))
        # bwd gather: [k, g, gs, b32, c, sig] -> [g, gs, k, sig, c, b32]
        g2 = Pc.reshape(Kd, 2, 2, 32, ns)[:, :, :, :, tb]
        lpb = np.ascontiguousarray(g2.transpose(1, 2, 0, 5, 4, 3))
        in_maps.append({"lpf": lpf, "lpb": lpb, "wts": wts, "cols": cols})
    return in_maps


def finish_output(results, ns, h_seg=16):
    NF = h_seg
    total = 0.0
    for res in results:
        z = np.asarray(res["zz"], np.float64)      # [2, 2, 29, 32]
        lnz = np.log(z)
        total += lnz[:, :, 0:NF - 1, :].sum() - lnz[:, :, NF - 1:, :].sum()
        total += 128 * ns * CHAT
    return np.float32(total)


# ---------------- harness entry point ----------------

_CACHED = {}


def _get_nc():
    if "nc" not in _CACHED:
        _CACHED["nc"] = build_nc()
    return _CACHED["nc"]


def kernel(log_pdf, pi, T, samples_per_sequence):
    """Full unsharded inputs -> full output (scalar f32), computed on 8
    TRN2 NeuronCores via the time-segmented scaled-forward kernel."""
    from concourse.bass_utils import run_bass_kernel_spmd

    ns = int(samples_per_sequence)
    assert log_pdf.shape == (64, 1048576) and ns == 1024, (
        "kernel is specialized to K=64, N=1048576, Ns=1024"
    )
    nc = _get_nc()
    in_maps = prep_inputs(np.asarray(log_pdf, np.float32),
                          np.asarray(pi, np.float32),
                          np.asarray(T, np.float32),
                          ns, h_seg=16, tc_chunk=8, jstar=8, n_cores=8)
    res = run_bass_kernel_spmd(nc, in_maps, core_ids=list(range(8)))
    return np.asarray(finish_output(res.results, ns, h_seg=16), np.float32)


# revision 14
# speedup vs baseline: 1.5911x; 1.0188x over previous
"""HMM forward kernel v3 — host-exp, H=16 segments, truncated backward
chains, DVE+Pool multiply split, merged matmuls.

Per core: 128 sequences x Ns=1024 steps, K=64 states.  Time is split into
H=16 segments of S=64 steps; products of positive matrices contract to
rank-1 (Birkhoff), so
  ll = ln(x0 . y_1) + sum_{h=1..H-2} [ln(w_h . y_{h+1}) - ln(w_h . 1)]
with x0 the exact seg-0 forward state, y_h = M_h @ 1 forward chains, and
w_h = M_h^T @ 1 backward chains.  w_h only matters in DIRECTION (its scale
cancels), and the chain contracts per step, so the backward recursion is
truncated to the last jstar=8 factors of each segment — measured total
rel err ~1e-4 vs the 2e-2 gate.

Host-side prep computes P = exp(lp - CHAT) into fp16 (no on-chip exp; ACT
idle).  The per-slot elementwise P-multiply splits across DVE (fwd chains
0..DF, bwd 0..DB) and Pool/gpsimd (the rest) to beat DVE's 1x f32-PSUM
throughput wall.  Matmuls are merged per weight (one 512-col fwd matmul,
one 448-col bwd matmul) so PE.SEQ dispatch is off the critical path.

Layouts (per stagger-group g of 64 seqs; partition = 64*gs + k):
  state  s   (128p, 30, 32)  bf16  chains = [fwd 0..16 | bwd 16..30]
  psum   v   (128p, 30, 32)  f32   fwd->bank0 (2048B exact), bwd->bank1
  chunks     (128p, TCc, 16, 32) f16  t-major so per-slot slices are
             contiguous; DRAM is partition-major so each chunk is ONE DMA
             of 128 x (TCc*16*32*2)B contiguous runs.
Backward chains use the pre-multiplied form w~_s = P_t(s) * (Tp @ w~_{s-1})
(host packs their P time-reversed); after slot jstar-1 they finish with one
bare matmul w = Tp @ w~ absorbed into slot jstar, and the stitch dot
products run via block-ones colsum matmuls at the end, with ln + final sum
on the host (output = 29 z-values per sequence).
"""

from contextlib import ExitStack

import numpy as np
import ml_dtypes

import concourse.bass as bass
import concourse.tile as tile
from concourse import bacc, mybir

F32 = mybir.dt.float32
F16 = mybir.dt.float16
BF16 = mybir.dt.bfloat16

K = 64
CHAT = 0.5


def build_nc(ns=1024, h_seg=16, tc_chunk=8, jstar=6, df=12, db=10,
             trn_type="TRN2"):
    S = ns // h_seg
    NF = h_seg               # fwd chains incl seg0
    NB = h_seg - 2           # bwd chains (w_{H-1} cancels)
    NCH = NF + NB
    n_chunks = S // tc_chunk
    assert jstar <= tc_chunk, "bwd data must fit chunk 0"
    nc = bacc.Bacc(trn_type, target_bir_lowering=False, debug=False)

    lp_f = nc.dram_tensor("lpf", [2, n_chunks, 2, K, tc_chunk, NF, 32],
                          F16, kind="ExternalInput")
    lp_b = nc.dram_tensor("lpb", [2, 2, K, jstar, NB, 32],
                          F16, kind="ExternalInput")
    wts = nc.dram_tensor("wts", [3, 128, 128], BF16, kind="ExternalInput")
    cols = nc.dram_tensor("cols", [2, 128], F32, kind="ExternalInput")
    out_z = nc.dram_tensor("zz", [2, 2, NCH - 1, 32], F32,
                           kind="ExternalOutput")

    with tile.TileContext(nc) as tc:
        with ExitStack() as ctx:
            _emit(ctx, tc, lp_f.ap(), lp_b.ap(), wts.ap(), cols.ap(),
                  out_z.ap(), S=S, NF=NF, NB=NB, NCH=NCH, TCc=tc_chunk,
                  n_chunks=n_chunks, jstar=jstar, DF=df, DB=db)
    nc.compile()
    return nc


def _emit(ctx, tc, lp_f, lp_b, wts, cols, out_z, *, S, NF, NB, NCH, TCc,
          n_chunks, jstar, DF, DB):
    nc = tc.nc

    consts = ctx.enter_context(tc.tile_pool(name="consts", bufs=1))
    pchf_pools = [ctx.enter_context(tc.tile_pool(name=f"pchf{g}", bufs=2))
                  for g in (0, 1)]
    pchb_pools = [ctx.enter_context(tc.tile_pool(name=f"pchb{g}", bufs=1))
                  for g in (0, 1)]
    sd_pools = [ctx.enter_context(tc.tile_pool(name=f"sd{g}", bufs=4))
                for g in (0, 1)]
    sp_pools = [ctx.enter_context(tc.tile_pool(name=f"sp{g}", bufs=4))
                for g in (0, 1)]
    small = ctx.enter_context(tc.tile_pool(name="small", bufs=2))
    up_pools = [ctx.enter_context(tc.tile_pool(name=f"up{g}", bufs=2))
                for g in (0, 1)]
    vd_pools = [ctx.enter_context(
        tc.tile_pool(name=f"vd{g}", bufs=1, space="PSUM"))
        for g in (0, 1)]
    vp_pools = [ctx.enter_context(
        tc.tile_pool(name=f"vp{g}", bufs=1, space="PSUM"))
        for g in (0, 1)]
    z_psum = ctx.enter_context(tc.tile_pool(name="zp", bufs=1, space="PSUM"))

    w_t = consts.tile([128, 3, 128], BF16, name="wt3")
    nc.sync.dma_start(w_t[:, 0, :], wts[0])
    nc.sync.dma_start(w_t[:, 1, :], wts[1])
    nc.sync.dma_start(w_t[:, 2, :], wts[2])
    WF, WB, WZ = w_t[:, 0, :], w_t[:, 1, :], w_t[:, 2, :]

    cols_t = consts.tile([128, 2], F32, name="cols_t")
    nc.sync.dma_start(cols_t[:, :], cols.rearrange("c p -> p c"))
    PIP, TAU = cols_t[:, 0:1], cols_t[:, 1:2]

    # finished backward vectors w_h = Tp @ w~ (written once at slot jstar)
    wfin = [consts.tile([128, NB, 32], BF16, name=f"wfin{g}") for g in (0, 1)]

    def load_fwd(g, i, t0, t1, t_=None):
        if t_ is None:
            t_ = pchf_pools[g].tile([128, TCc, NF, 32], F16, name="pchf",
                                    tag="pchf")
        eng = nc.sync if (g + i) % 2 == 0 else nc.scalar
        eng.dma_start(
            t_[:, t0:t1, :, :],
            lp_f[g, i, :, :, t0:t1].rearrange("gs k t c b -> (gs k) t c b"),
        )
        return t_

    # chunk-0 + bwd loads, split in t-halves so group 0 starts early
    hf = TCc // 2
    hb = jstar // 2
    pch = [None, None]
    pchb = [None, None]
    for g in (0, 1):
        pchb[g] = pchb_pools[g].tile([128, jstar, NB, 32], F16, name="pchb",
                                     tag="pchb")
    for t0, t1, b0, b1 in ((0, hf, 0, hb), (hf, TCc, hb, jstar)):
        for g in (0, 1):
            pch[g] = load_fwd(g, 0, t0, t1, t_=pch[g])
            eng = nc.scalar if g == 0 else nc.sync
            eng.dma_start(
                pchb[g][:, b0:b1, :, :],
                lp_b[g, :, :, b0:b1].rearrange("gs k t c b -> (gs k) t c b"),
            )

    # ---- slot 0: chain inits ----
    # Per-engine state ownership: DVE owns fwd chains 0:DF + bwd 0:DB in
    # s_d; Pool owns fwd DF:NF + bwd DB:NB in s_p.  No tile is ever
    # co-written by two engines, so each pipeline ping-pongs on one
    # semaphore pair with the PE and the groups overlap freely.
    PF = NF - DF
    PB = NB - DB
    sd = [None, None]
    sp = [None, None]
    for g in (0, 1):
        s0d = sd_pools[g].tile([128, DF + DB, 32], BF16, name="sd", tag="sd")
        p0 = pch[g][:, 0, :, :]
        nc.vector.tensor_scalar_mul(s0d[:, 0, :], p0[:, 0, :], PIP)
        nc.vector.tensor_scalar_mul(s0d[:, 1:DF, :], p0[:, 1:DF, :], TAU)
        nc.vector.tensor_copy(s0d[:, DF:DF + DB, :], pchb[g][:, 0, 0:DB, :])
        sd[g] = s0d
        s0p = sp_pools[g].tile([128, PF + PB, 32], BF16, name="sp", tag="sp")
        nc.gpsimd.tensor_scalar_mul(s0p[:, 0:PF, :], p0[:, DF:NF, :], TAU)
        nc.gpsimd.tensor_copy(s0p[:, PF:PF + PB, :], pchb[g][:, 0, DB:NB, :])
        sp[g] = s0p

    # ---- main slot loop ----
    # Matmuls are split by the engine that owns the chains (DVE: fwd 0:DF,
    # bwd NF:NF+DB; Pool: the rest) so every matmul and every multiply
    # carries exactly ONE inline sync wait — the two engine pipelines stay
    # fully decoupled and the stagger groups overlap cleanly.
    nxt = None
    for sig in range(1, S):
        i = sig // TCc
        t = sig % TCc
        if t == 1 and i + 1 < n_chunks:
            nxt = [load_fwd(g, i + 1, 0, TCc) for g in (0, 1)]
        if t == 0 and i > 0:
            pch = nxt
        bwd = sig < jstar
        vvd = [None, None]
        vvp = [None, None]
        for g in (0, 1):
            # v_d: fwd at [0:DF] (bank0), bwd at [16:16+DB] (bank1);
            # v_p: fwd+bwd packed in one bank
            v_d = vd_pools[g].tile([128, 26, 32], F32, name="vd", tag="vd")
            v_p = vp_pools[g].tile([128, PF + PB, 32], F32, name="vp",
                                   tag="vp")
            vvd[g], vvp[g] = v_d, v_p
            nc.tensor.matmul(v_d[:, 0:DF, :], lhsT=WF, rhs=sd[g][:, 0:DF, :],
                             start=True, stop=True)
            nc.tensor.matmul(v_p[:, 0:PF, :], lhsT=WF, rhs=sp[g][:, 0:PF, :],
                             start=True, stop=True)
        if bwd or sig == jstar:
            for g in (0, 1):
                # sig==jstar: bare finishing matmul w = Tp @ w~
                nc.tensor.matmul(vvd[g][:, 16:16 + DB, :], lhsT=WB,
                                 rhs=sd[g][:, DF:DF + DB, :], start=True,
                                 stop=True)
                nc.tensor.matmul(vvp[g][:, PF:PF + PB, :], lhsT=WB,
                                 rhs=sp[g][:, PF:PF + PB, :], start=True,
                                 stop=True)
        for g in (0, 1):
            v_d, v_p = vvd[g], vvp[g]
            s_nd = sd_pools[g].tile([128, DF + DB, 32], BF16, name="sd",
                                    tag="sd")
            s_np = sp_pools[g].tile([128, PF + PB, 32], BF16, name="sp",
                                    tag="sp")
            # ACT stages v_p into SBUF (GPSIMD cannot read PSUM); Pool
            # multiplies SBUF-only, keeping those chains off DVE
            u_p = up_pools[g].tile([128, PF + PB, 32], BF16, name="up",
                                   tag="up")
            ce = PF + PB if bwd else PF
            nc.scalar.copy(u_p[:, 0:ce, :], v_p[:, 0:ce, :])
            if bwd:
                nc.vector.tensor_mul(s_nd[:, DF:DF + DB, :],
                                     v_d[:, 16:16 + DB, :],
                                     pchb[g][:, sig, 0:DB, :])
                nc.gpsimd.tensor_mul(s_np[:, PF:PF + PB, :],
                                     u_p[:, PF:PF + PB, :],
                                     pchb[g][:, sig, DB:NB, :])
            elif sig == jstar:
                nc.vector.tensor_copy(wfin[g][:, 0:DB, :],
                                      v_d[:, 16:16 + DB, :])
                nc.vector.tensor_copy(wfin[g][:, DB:NB, :],
                                      v_p[:, PF:PF + PB, :])
            nc.vector.tensor_mul(s_nd[:, 0:DF, :], v_d[:, 0:DF, :],
                                 pch[g][:, t, 0:DF, :])
            nc.gpsimd.tensor_mul(s_np[:, 0:PF, :], u_p[:, 0:PF, :],
                                 pch[g][:, t, DF:NF, :])
            sd[g], sp[g] = s_nd, s_np

    # ---- stitch ----
    for g in (0, 1):
        dp = small.tile([128, NF - 1, 32], BF16, name="dp", tag="dp")
        nc.vector.tensor_mul(dp[:, 0:1, :], sd[g][:, 0:1, :],
                             sd[g][:, 1:2, :])
        nc.vector.tensor_mul(dp[:, 1:DF - 1, :], wfin[g][:, 0:DF - 2, :],
                             sd[g][:, 2:DF, :])
        nc.vector.tensor_mul(dp[:, DF - 1:NF - 1, :],
                             wfin[g][:, DF - 2:NB, :], sp[g][:, 0:PF, :])
        zz = z_psum.tile([128, 32, 32], F32, name="zz", tag="zz")
        nc.tensor.matmul(zz[:, 0:NF - 1, :], lhsT=WZ, rhs=dp[:, :, :],
                         start=True, stop=True)
        nc.tensor.matmul(zz[:, 16:16 + DB, :], lhsT=WZ,
                         rhs=wfin[g][:, 0:DB, :], start=True, stop=True)
        nc.tensor.matmul(zz[:, 16 + DB:16 + NB, :], lhsT=WZ,
                         rhs=wfin[g][:, DB:NB, :], start=True, stop=True)
        zs = small.tile([128, NCH - 1, 32], F32, name="zs", tag="zs")
        nc.vector.tensor_copy(zs[:, 0:NF - 1, :], zz[:, 0:NF - 1, :])
        nc.vector.tensor_copy(zs[:, NF - 1:NCH - 1, :], zz[:, 16:16 + NB, :])
        for gs in (0, 1):
            p0 = 64 * gs
            nc.sync.dma_start(out_z[g, gs:gs + 1, :, :],
                              zs[p0:p0 + 1, :, :])


# ---------------- host side ----------------

def _log_softmax(x, axis):
    x = np.asarray(x, np.float64)
    m = x.max(axis=axis, keepdims=True)
    return x - m - np.log(np.exp(x - m).sum(axis=axis, keepdims=True))


def prep_inputs(log_pdf, pi, T, ns, h_seg=16, tc_chunk=8, jstar=6,
                n_cores=8):
    Kd, N = log_pdf.shape
    b_total = N // ns
    b_core = b_total // n_cores
    S = ns // h_seg
    NF = h_seg
    NB = h_seg - 2
    n_chunks = S // tc_chunk

    logT = _log_softmax(T, 1)
    Tp = np.exp(logT)
    logpi = _log_softmax(pi, 0)

    wf = np.zeros((128, 128), np.float64)
    wf[:64, :64] = Tp; wf[64:, 64:] = Tp
    wb = np.zeros((128, 128), np.float64)
    wb[:64, :64] = Tp.T; wb[64:, 64:] = Tp.T
    wz = np.zeros((128, 128), np.float64)
    wz[:64, :64] = 1.0; wz[64:, 64:] = 1.0
    wts = np.stack([wf, wb, wz]).astype(ml_dtypes.bfloat16)

    pip = np.exp(logpi)
    tau = Tp.sum(axis=0)
    cols = np.stack([np.concatenate([pip, pip]),
                     np.concatenate([tau, tau])]).astype(np.float32)

    # host-side exp: P = exp(lp - CHAT) in fp16
    P_all = np.exp(np.asarray(log_pdf, np.float32) - CHAT).astype(np.float16)

    # bwd time map: chain c (seg h=c+1), slot sig -> t = (c+2)S - 1 - sig
    tb = (np.arange(NB)[:, None] + 2) * S - 1 - np.arange(jstar)[None, :]

    in_maps = []
    for core in range(n_cores):
        Pc = P_all[:, core * b_core * ns:(core + 1) * b_core * ns]
        Pc = Pc.reshape(Kd, b_core, ns)            # [k, b, t]
        # fwd: t = c*S + i*TCc + tt ; b = 64g + 32gs + b32
        v = Pc.reshape(Kd, 2, 2, 32, NF, n_chunks, tc_chunk)
        lpf = np.ascontiguousarray(v.transpose(1, 5, 2, 0, 6, 4, 3))
        # bwd gather: [k, g, gs, b32, c, sig] -> [g, gs, k, sig, c, b32]
        g2 = Pc.reshape(Kd, 2, 2, 32, ns)[:, :, :, :, tb]
        lpb = np.ascontiguousarray(g2.transpose(1, 2, 0, 5, 4, 3))
        in_maps.append({"lpf": lpf, "lpb": lpb, "wts": wts, "cols": cols})
    return in_maps


def finish_output(results, ns, h_seg=16):
    NF = h_seg
    total = 0.0
    for res in results:
        z = np.asarray(res["zz"], np.float64)      # [2, 2, 29, 32]
        lnz = np.log(z)
        total += lnz[:, :, 0:NF - 1, :].sum() - lnz[:, :, NF - 1:, :].sum()
        total += 128 * ns * CHAT
    return np.float32(total)


# ---------------- harness entry point ----------------

_CACHED = {}


def _get_nc():
    if "nc" not in _CACHED:
        _CACHED["nc"] = build_nc()
    return _CACHED["nc"]


def kernel(log_pdf, pi, T, samples_per_sequence):
    """Full unsharded inputs -> full output (scalar f32), computed on 8
    TRN2 NeuronCores via the time-segmented scaled-forward kernel."""
    from concourse.bass_utils import run_bass_kernel_spmd

    ns = int(samples_per_sequence)
    assert log_pdf.shape == (64, 1048576) and ns == 1024, (
        "kernel is specialized to K=64, N=1048576, Ns=1024"
    )
    nc = _get_nc()
    in_maps = prep_inputs(np.asarray(log_pdf, np.float32),
                          np.asarray(pi, np.float32),
                          np.asarray(T, np.float32),
                          ns, h_seg=16, tc_chunk=8, jstar=6, n_cores=8)
    res = run_bass_kernel_spmd(nc, in_maps, core_ids=list(range(8)))
    return np.asarray(finish_output(res.results, ns, h_seg=16), np.float32)
